# revision 40
# baseline (speedup 1.0000x reference)
"""MSRSA multi-head attention kernel for 8 Trainium2 NeuronCores.

Strategy: data-parallel over batch (B=8 -> 1 batch element per core).
Per core, for its batch element b:
  Qt = (W_q/8) @ queries^T        [512,1024]  (scale 1/8 folded into W_q)
  Kt = W_k @ keys^T               [512,1024]
  V  = values @ W_v^T             [1024,512]  (rows masked by attention_mask)
  per head h, scores are computed TRANSPOSED: S_T[k,q]:
     S_T = sum_d Kt[d,k]*Qt[d,q] + la[h]*A^T[k,q] + ld[h]*D^T[k,q]
  (A/D bias injected by scaled-identity matmuls accumulating into PSUM)
  expS = exp(S_T) on ScalarE (PSUM -> SBUF evacuation is the exp)
  attnT_h[d,q] (+ denominator row) = sum_k V_ext[k, d|mask] * expS[k,q]
  (mask column of V_ext -> row 64 of PV output = softmax denominator)
  normalize: denom row copied to partition 0 (copies may retarget the output
  base), reciprocal_approx_fast at base 0, fp16 cast, K=1 ones-matmul
  broadcast; the multiply writes even heads at partitions 0:64 and odd heads
  at 64:128 directly.
  out = attnT contracted with W_o^T   [1024, 512] (fp16, host upcasts)

On TRN2 a matmul costs ~N output columns at 1 col/cycle regardless of dtype
or contraction size, so the kernel minimizes matmul COUNT and keeps weight
loads small (fp16 64-col ident tiles) so LDWEIGHTS hides under execution.
"""

import contextlib

import numpy as np

import concourse.bass as bass
import concourse.mybir as mybir
import concourse.tile as tile
from concourse.bass_utils import run_bass_kernel_spmd

B, L, DIN, DM, H = 8, 1024, 256, 512, 8
DH = DM // H  # 64
P = 128
NKT = L // P          # 8 k-tiles
NQC = 2               # q chunks
QC = L // NQC         # 512
F32 = mybir.dt.float32
F16 = mybir.dt.float16
F8 = mybir.dt.float8e4
DR = mybir.MatmulPerfMode.DoubleRow


def _emit(tc):
    nc = tc.nc

    def dram(name, shape, dtype=F16, kind="ExternalInput"):
        return nc.dram_tensor(name, shape, dtype, kind=kind).ap()

    qT = dram("qT", [DIN, L])
    kT = dram("kT", [DIN, L])
    vT = dram("vT", [DIN, L])
    wqT = dram("wqT", [DIN, DM])
    wkT = dram("wkT", [DIN, DM])
    wvT = dram("wvT", [DIN, DM])
    woT = dram("woT", [DM, DM])
    ad8 = dram("ad8", [NKT * P, 2 * L], F8)  # per row: [A | fp8(D-5)]
    idn8 = dram("idn8", [P, H * 2 * P], F8)  # per-head (la, ld) diag subtiles
    mask01 = dram("mask01", [P, NKT], F32)
    out = dram("out", [L, DM], F16, kind="ExternalOutput")

    with contextlib.ExitStack() as ctx:
        singles = ctx.enter_context(tc.tile_pool(name="singles", bufs=1))
        big = ctx.enter_context(tc.tile_pool(name="big", bufs=1))
        exps = ctx.enter_context(tc.tile_pool(name="exps", bufs=4))
        small = ctx.enter_context(tc.tile_pool(name="small", bufs=3))
        attnp = ctx.enter_context(tc.tile_pool(name="attnp", bufs=2))
        spsum = ctx.enter_context(tc.tile_pool(name="spsum", bufs=2, space="PSUM"))
        pvwo = ctx.enter_context(tc.tile_pool(name="pvwo", bufs=3, space="PSUM"))
        bcp = ctx.enter_context(tc.tile_pool(name="bcp", bufs=1, space="PSUM"))

        # ---- big SBUF-resident tensors ----
        ad_sb = big.tile([P, NKT, 2, L], F8, tag="ad")  # [A | Dhi] k-tile rows
        qt_sb = big.tile([P, 4, L], F16, tag="qt")       # [p,t,l] = Qt[t*128+p, l]
        kt_sb = big.tile([P, 4, L], F16, tag="kt")
        vx_sb = big.tile([P, NKT, H, DH + 1], F16, tag="vx")  # V + mask column

        ad8_r = ad8.rearrange("(t p) (j q) -> p t j q", p=P, q=L)

        # ---- phase 1: projections (pools scoped so SBUF is reclaimed) ----
        proj_ctx = contextlib.ExitStack()
        stage = proj_ctx.enter_context(tc.tile_pool(name="stage", bufs=3))
        wpool = proj_ctx.enter_context(tc.tile_pool(name="wpool", bufs=3))

        def load_stage(src, eng):
            t = stage.tile([P, 2, L], F16, tag="stage")
            r = src.rearrange("(t p) l -> p t l", p=P)
            for i in range(2):  # per-half DMAs so the first matmul starts early
                eng.dma_start(out=t[:, i, :], in_=r[:, i, :])
            return t

        def load_w(src, eng):
            t = wpool.tile([P, 2, DM], F16, tag="w")
            r = src.rearrange("(t p) d -> p t d", p=P)
            for i in range(2):
                eng.dma_start(out=t[:, i, :], in_=r[:, i, :])
            return t

        # DMA issue order = dependency order: Q/K paths gate the first
        # matmuls, idents+ad gate the first bias matmul, V/Wo come later.
        # Issue across both HWDGE engines (sync + scalar) so descriptor
        # generation is not serialized at the head of the kernel.
        q_sb, wq_sb = load_stage(qT, nc.sync), load_w(wqT, nc.scalar)
        k_sb, wk_sb = load_stage(kT, nc.sync), load_w(wkT, nc.scalar)

        idents = singles.tile([P, H, 2, P], F8, tag="idents")
        nc.scalar.dma_start(
            out=idents[:], in_=idn8.rearrange("p (h j m) -> p h j m", h=H, j=2)
        )
        mask_sb = singles.tile([P, NKT], F32, tag="mask")
        nc.sync.dma_start(out=mask_sb[:], in_=mask01[:])
        # first two ad tiles gate the first bias matmuls; the rest can land
        # while the early heads run
        for t in range(2):
            eng = nc.sync if t % 2 == 0 else nc.scalar
            eng.dma_start(out=ad_sb[:, t, :, :], in_=ad8_r[:, t, :, :])

        ones_sb = singles.tile([P, DH], F16, tag="ones")
        nc.vector.memset(ones_sb[:], 1.0)

        v_sb, wv_sb = load_stage(vT, nc.sync), load_w(wvT, nc.scalar)
        for t in range(2, NKT):
            eng = nc.sync if t % 2 == 0 else nc.scalar
            eng.dma_start(out=ad_sb[:, t, :, :], in_=ad8_r[:, t, :, :])
        wo_sb = singles.tile([P, 4, DM], F16, tag="wo")
        nc.sync.dma_start(out=wo_sb[:], in_=woT.rearrange("(t p) d -> p t d", p=P))

        # Qt / Kt: out[m=dm-tile, n=l-chunk] = sum_din w?T[din, dm] * xT[din, l]
        for x_sb, w_sb, dst in ((q_sb, wq_sb, qt_sb), (k_sb, wk_sb, kt_sb)):
            for mt in range(4):
                for lc in range(NQC):
                    ps = pvwo.tile([P, QC], F32, tag="pvwo")
                    for kt2 in range(2):
                        nc.tensor.matmul(
                            ps[:],
                            w_sb[:, kt2, mt * P : (mt + 1) * P],
                            x_sb[:, kt2, lc * QC : (lc + 1) * QC],
                            start=(kt2 == 0),
                            stop=(kt2 == 1),
                        )
                    nc.vector.tensor_copy(
                        out=dst[:, mt, lc * QC : (lc + 1) * QC], in_=ps[:]
                    )

        # V: out[m=l-tile, n=dm] = sum_din vT[din, l] * wvT[din, dm]; mask rows
        for lt in range(NKT):
            ps = pvwo.tile([P, DM], F32, tag="pvwo")
            for kt2 in range(2):
                nc.tensor.matmul(
                    ps[:],
                    v_sb[:, kt2, lt * P : (lt + 1) * P],
                    wv_sb[:, kt2, :],
                    start=(kt2 == 0),
                    stop=(kt2 == 1),
                )
            nc.vector.tensor_scalar_mul(
                out=vx_sb[:, lt, :, 0:DH],
                in0=ps.rearrange("p (h d) -> p h d", h=H),
                scalar1=mask_sb[:, lt : lt + 1],
            )
            # mask column (softmax denominator counts only unmasked keys)
            nc.vector.tensor_copy(
                out=vx_sb[:, lt, :, DH : DH + 1],
                in_=mask_sb[:, lt : lt + 1, None].to_broadcast((P, H, 1)),
            )

        proj_ctx.close()

        # ---- phase 2: attention ----
        # qc0's output projection is emitted after qc1's first head so the PE
        # has score work queued while the last head's normalization (vector
        # side) completes — attnT is double-buffered to allow it.
        attnT_tiles = {}

        def emit_outproj(qc):
            attn = attnT_tiles[qc]
            for lt in range(QC // P):
                ws = pvwo.tile([P, DM], F32, tag="pvwo")
                for kt4 in range(4):
                    nc.tensor.matmul(
                        ws[:],
                        attn[:, kt4, lt * P : (lt + 1) * P],
                        wo_sb[:, kt4, :],
                        start=(kt4 == 0),
                        stop=(kt4 == 3),
                    )
                ost = small.tile([P, DM], F16, tag="ost")
                nc.scalar.copy(out=ost[:], in_=ws[:])
                nc.sync.dma_start(
                    out=out[qc * QC + lt * P : qc * QC + (lt + 1) * P, :],
                    in_=ost[:],
                )

        for qc in range(NQC):
            qs = slice(qc * QC, (qc + 1) * QC)
            attnT_sb = attnp.tile([P, 4, QC], F16, tag="attnT")
            attnT_tiles[qc] = attnT_sb
            for h in range(H):
                hb = (h % 2) * DH  # partition base of head h inside its dm-tile
                ht = h // 2
                ex = exps.tile([P, NKT, QC], F16, tag="ex")
                for ktp in range(NKT // 2):  # pairs of k-tiles share a psum
                    sp = spsum.tile([P, 2 * QC], F32, tag="sp")
                    # per k-tile: one fp8 DoubleRow bias matmul (la*A +
                    # ld*fp8(D-5), starts the PSUM region) then the score;
                    # interleaving keeps each LDWEIGHTS under the other's
                    # execution
                    for i in range(2):
                        kt = 2 * ktp + i
                        nc.tensor.matmul(
                            sp[:, i * QC : (i + 1) * QC],
                            idents[:, h, :, :], ad_sb[:, kt, :, qs],
                            start=True, stop=False, perf_mode=DR,
                        )
                        nc.tensor.matmul(
                            sp[:, i * QC : (i + 1) * QC],
                            kt_sb[hb : hb + DH, ht, kt * P : (kt + 1) * P],
                            qt_sb[hb : hb + DH, ht, qs],
                            start=False,
                            stop=True,
                        )
                    nc.scalar.activation(
                        out=ex[:, 2 * ktp : 2 * ktp + 2, :].rearrange(
                            "p a b -> p (a b)"
                        ),
                        in_=sp[:],
                        func=mybir.ActivationFunctionType.Exp,
                    )
                # PV with appended mask column -> row 64 = softmax denominator
                pv = pvwo.tile([P, QC], F32, tag="pvwo")
                for kt in range(NKT):
                    nc.tensor.matmul(
                        pv[0 : DH + 1, :],
                        vx_sb[:, kt, h, :],
                        ex[:, kt, :],
                        start=(kt == 0),
                        stop=(kt == NKT - 1),
                    )
                # normalize: shift denom row to partition 0, fast recip, fp16
                # cast, K=1 ones-matmul broadcast across 64 partitions
                den = small.tile([1, QC], F32, tag="den")
                nc.vector.tensor_copy(out=den[:], in_=pv[DH : DH + 1, :])
                rec = small.tile([1, QC], F32, tag="rec")
                nc.vector.reciprocal_approx_fast(out=rec[:], in_=den[:])
                rec16 = small.tile([1, QC], F16, tag="rec16")
                nc.vector.tensor_copy(out=rec16[:], in_=rec[:])
                bps = bcp.tile([DH, QC], F32, tag="bps")
                nc.tensor.matmul(
                    bps[:],
                    ones_sb[0:1, :],
                    rec16[:],
                    start=True,
                    stop=True,
                )
                pvs = small.tile([DH, QC], F32, tag="bc")
                nc.vector.tensor_copy(out=pvs[:], in_=pv[0:DH, :])
                # inputs share base 0; output base may differ (odd heads land
                # on partitions 64:128 directly)
                nc.vector.tensor_mul(
                    out=attnT_sb[hb : hb + DH, ht, :], in0=pvs[:], in1=bps[:]
                )
                if qc == 1 and h == 0:
                    emit_outproj(0)

        emit_outproj(1)


def build_nc():
    from concourse import bacc

    nc = bacc.Bacc("TRN2", target_bir_lowering=False, debug=False)
    with tile.TileContext(nc) as tc:
        _emit(tc)
    nc.compile()
    return nc


_NC = None


def _get_nc():
    global _NC
    if _NC is None:
        _NC = build_nc()
    return _NC


def make_in_maps(queries, keys, values, attention_mask, adjacency_matrix,
                 distance_matrix, W_q, W_k, W_v, W_o, lambda_a, lambda_d):
    import ml_dtypes

    f = np.float32
    h16 = np.float16
    f8 = ml_dtypes.float8_e4m3
    c = np.ascontiguousarray
    wqT = c((W_q.astype(f) * f(0.125)).T).astype(h16)
    wkT = c(W_k.astype(f).T).astype(h16)
    wvT = c(W_v.astype(f).T).astype(h16)
    woT = c(W_o.astype(f).T).astype(h16)
    la8 = lambda_a.astype(f).astype(f8).astype(f)
    ld8 = lambda_d.astype(f).astype(f8).astype(f)
    idn = np.zeros((P, H, 2, P), dtype=f)
    rr = np.arange(P)
    for h in range(H):
        idn[rr, h, 0, rr] = la8[h]
        idn[rr, h, 1, rr] = ld8[h]
    idn8 = idn.reshape(P, H * 2 * P).astype(f8)
    in_maps = []
    for b in range(B):
        # per k-tile block of 128 rows: [A | fp8(D-5)]; the -5 shift centers
        # D's fp8 range and cancels in softmax
        A8 = adjacency_matrix[b].astype(f).T.astype(f8)
        Dhi = (distance_matrix[b].astype(f).T - f(5.0)).astype(f8)
        ad = np.concatenate(
            [A8.reshape(NKT, P, L), Dhi.reshape(NKT, P, L)], axis=2
        )  # [NKT, P, 2L]
        in_maps.append({
            "qT": c(queries[b].astype(f).T).astype(h16),
            "kT": c(keys[b].astype(f).T).astype(h16),
            "vT": c(values[b].astype(f).T).astype(h16),
            "wqT": wqT, "wkT": wkT, "wvT": wvT, "woT": woT,
            "ad8": c(ad.reshape(NKT * P, 2 * L)),
            "mask01": c((attention_mask[b] > 0).astype(f).reshape(NKT, P).T),
            "idn8": idn8,
        })
    return in_maps


def kernel(queries, keys, values, attention_mask, adjacency_matrix,
           distance_matrix, W_q, W_k, W_v, W_o, lambda_a, lambda_d, **kw):
    nc = _get_nc()
    in_maps = make_in_maps(queries, keys, values, attention_mask,
                           adjacency_matrix, distance_matrix,
                           W_q, W_k, W_v, W_o, lambda_a, lambda_d)
    res = run_bass_kernel_spmd(nc, in_maps, list(range(B)), **kw)
    outs = np.stack([res.results[i]["out"] for i in range(B)]).astype(np.float32)
    return outs


# revision 41
# speedup vs baseline: 1.0424x; 1.0424x over previous
"""MSRSA multi-head attention kernel for 8 Trainium2 NeuronCores.

Strategy: data-parallel over batch (B=8 -> 1 batch element per core).
Per core, for its batch element b:
  Qt = (W_q/8) @ queries^T        [512,1024]  (scale 1/8 folded into W_q)
  Kt = W_k @ keys^T               [512,1024]
  V  = values @ W_v^T             [1024,512]  (rows masked by attention_mask)
  per head h, scores are computed TRANSPOSED: S_T[k,q]:
     S_T = sum_d Kt[d,k]*Qt[d,q] + la[h]*A^T[k,q] + ld[h]*D^T[k,q]
  (A/D bias injected by scaled-identity matmuls accumulating into PSUM)
  expS = exp(S_T) on ScalarE (PSUM -> SBUF evacuation is the exp)
  attnT_h[d,q] (+ denominator row) = sum_k V_ext[k, d|mask] * expS[k,q]
  (mask column of V_ext -> row 64 of PV output = softmax denominator)
  normalize: denom row copied to partition 0 (copies may retarget the output
  base), reciprocal_approx_fast at base 0, fp16 cast, K=1 ones-matmul
  broadcast; the multiply writes even heads at partitions 0:64 and odd heads
  at 64:128 directly.
  out = attnT contracted with W_o^T   [1024, 512] (fp16, host upcasts)

On TRN2 a matmul costs ~N output columns at 1 col/cycle regardless of dtype
or contraction size, so the kernel minimizes matmul COUNT and keeps weight
loads small (fp16 64-col ident tiles) so LDWEIGHTS hides under execution.
"""

import contextlib

import numpy as np

import concourse.bass as bass
import concourse.mybir as mybir
import concourse.tile as tile
from concourse.bass_utils import run_bass_kernel_spmd

B, L, DIN, DM, H = 8, 1024, 256, 512, 8
DH = DM // H  # 64
P = 128
NKT = L // P          # 8 k-tiles
NQC = 2               # q chunks
QC = L // NQC         # 512
F32 = mybir.dt.float32
F16 = mybir.dt.float16
F8 = mybir.dt.float8e4
DR = mybir.MatmulPerfMode.DoubleRow


def _emit(tc):
    nc = tc.nc

    def dram(name, shape, dtype=F16, kind="ExternalInput"):
        return nc.dram_tensor(name, shape, dtype, kind=kind).ap()

    qT = dram("qT", [DIN, L])
    kT = dram("kT", [DIN, L])
    vT = dram("vT", [DIN, L])
    wqT = dram("wqT", [DIN, DM])
    wkT = dram("wkT", [DIN, DM])
    wvT = dram("wvT", [DIN, DM])
    woT = dram("woT", [DM, DM])
    ad8 = dram("ad8", [NKT * P, 2 * L], F8)  # per row: [A | fp8(D-5)]
    idn8 = dram("idn8", [P, H * 2 * P], F8)  # per-head (la, ld) diag subtiles
    mask01 = dram("mask01", [P, NKT], F32)
    out = dram("out", [L, DM], F16, kind="ExternalOutput")

    with contextlib.ExitStack() as ctx:
        singles = ctx.enter_context(tc.tile_pool(name="singles", bufs=1))
        big = ctx.enter_context(tc.tile_pool(name="big", bufs=1))
        exps = ctx.enter_context(tc.tile_pool(name="exps", bufs=4))
        small = ctx.enter_context(tc.tile_pool(name="small", bufs=3))
        attnp = ctx.enter_context(tc.tile_pool(name="attnp", bufs=2))
        spsum = ctx.enter_context(tc.tile_pool(name="spsum", bufs=2, space="PSUM"))
        pvwo = ctx.enter_context(tc.tile_pool(name="pvwo", bufs=3, space="PSUM"))
        bcp = ctx.enter_context(tc.tile_pool(name="bcp", bufs=1, space="PSUM"))

        # ---- big SBUF-resident tensors ----
        ad_sb = big.tile([P, NKT, 2, L], F8, tag="ad")  # [A | Dhi] k-tile rows
        qt_sb = big.tile([P, 4, L], F16, tag="qt")       # [p,t,l] = Qt[t*128+p, l]
        kt_sb = big.tile([P, 4, L], F16, tag="kt")
        vx_sb = big.tile([P, NKT, H, DH + 1], F16, tag="vx")  # V + mask column

        ad8_r = ad8.rearrange("(t p) (j q) -> p t j q", p=P, q=L)

        # ---- phase 1: projections (pools scoped so SBUF is reclaimed) ----
        proj_ctx = contextlib.ExitStack()
        stage = proj_ctx.enter_context(tc.tile_pool(name="stage", bufs=3))
        wpool = proj_ctx.enter_context(tc.tile_pool(name="wpool", bufs=3))

        def load_stage(src, eng):
            t = stage.tile([P, 2, L], F16, tag="stage")
            r = src.rearrange("(t p) l -> p t l", p=P)
            for i in range(2):  # per-half DMAs so the first matmul starts early
                eng.dma_start(out=t[:, i, :], in_=r[:, i, :])
            return t

        def load_w(src, eng):
            t = wpool.tile([P, 2, DM], F16, tag="w")
            r = src.rearrange("(t p) d -> p t d", p=P)
            for i in range(2):
                eng.dma_start(out=t[:, i, :], in_=r[:, i, :])
            return t

        # DMA issue order = dependency order: Q/K paths gate the first
        # matmuls, idents+ad gate the first bias matmul, V/Wo come later.
        # Issue across both HWDGE engines (sync + scalar) so descriptor
        # generation is not serialized at the head of the kernel.
        q_sb, wq_sb = load_stage(qT, nc.sync), load_w(wqT, nc.scalar)
        k_sb, wk_sb = load_stage(kT, nc.sync), load_w(wkT, nc.scalar)

        idents = singles.tile([P, H, 2, P], F8, tag="idents")
        nc.scalar.dma_start(
            out=idents[:], in_=idn8.rearrange("p (h j m) -> p h j m", h=H, j=2)
        )
        mask_sb = singles.tile([P, NKT], F32, tag="mask")
        nc.sync.dma_start(out=mask_sb[:], in_=mask01[:])
        # first two ad tiles gate the first bias matmuls; the rest can land
        # while the early heads run
        for t in range(2):
            eng = nc.sync if t % 2 == 0 else nc.scalar
            eng.dma_start(out=ad_sb[:, t, :, :], in_=ad8_r[:, t, :, :])

        ones_sb = singles.tile([P, DH], F16, tag="ones")
        nc.vector.memset(ones_sb[:], 1.0)

        v_sb, wv_sb = load_stage(vT, nc.sync), load_w(wvT, nc.scalar)
        for t in range(2, NKT):
            eng = nc.sync if t % 2 == 0 else nc.scalar
            eng.dma_start(out=ad_sb[:, t, :, :], in_=ad8_r[:, t, :, :])
        wo_sb = singles.tile([P, 4, DM], F16, tag="wo")
        nc.sync.dma_start(out=wo_sb[:], in_=woT.rearrange("(t p) d -> p t d", p=P))

        # Qt / Kt: out[m=dm-tile, n=l-chunk] = sum_din w?T[din, dm] * xT[din, l]
        for x_sb, w_sb, dst in ((q_sb, wq_sb, qt_sb), (k_sb, wk_sb, kt_sb)):
            for mt in range(4):
                for lc in range(NQC):
                    ps = pvwo.tile([P, QC], F32, tag="pvwo")
                    for kt2 in range(2):
                        nc.tensor.matmul(
                            ps[:],
                            w_sb[:, kt2, mt * P : (mt + 1) * P],
                            x_sb[:, kt2, lc * QC : (lc + 1) * QC],
                            start=(kt2 == 0),
                            stop=(kt2 == 1),
                        )
                    nc.vector.tensor_copy(
                        out=dst[:, mt, lc * QC : (lc + 1) * QC], in_=ps[:]
                    )

        # V: out[m=l-tile, n=dm] = sum_din vT[din, l] * wvT[din, dm]; mask rows
        for lt in range(NKT):
            ps = pvwo.tile([P, DM], F32, tag="pvwo")
            for kt2 in range(2):
                nc.tensor.matmul(
                    ps[:],
                    v_sb[:, kt2, lt * P : (lt + 1) * P],
                    wv_sb[:, kt2, :],
                    start=(kt2 == 0),
                    stop=(kt2 == 1),
                )
            nc.vector.tensor_scalar_mul(
                out=vx_sb[:, lt, :, 0:DH],
                in0=ps.rearrange("p (h d) -> p h d", h=H),
                scalar1=mask_sb[:, lt : lt + 1],
            )
            # mask column (softmax denominator counts only unmasked keys)
            nc.vector.tensor_copy(
                out=vx_sb[:, lt, :, DH : DH + 1],
                in_=mask_sb[:, lt : lt + 1, None].to_broadcast((P, H, 1)),
            )

        proj_ctx.close()

        # ---- phase 2: attention ----
        # qc0's output projection is emitted after qc1's first head so the PE
        # has score work queued while the last head's normalization (vector
        # side) completes — attnT is double-buffered to allow it.
        attnT_tiles = {}

        def emit_outproj(qc):
            attn = attnT_tiles[qc]
            for lt in range(QC // P):
                ws = pvwo.tile([P, DM], F32, tag="pvwo")
                for kt4 in range(4):
                    nc.tensor.matmul(
                        ws[:],
                        attn[:, kt4, lt * P : (lt + 1) * P],
                        wo_sb[:, kt4, :],
                        start=(kt4 == 0),
                        stop=(kt4 == 3),
                    )
                ost = small.tile([P, DM], F16, tag="ost")
                nc.scalar.copy(out=ost[:], in_=ws[:])
                nc.sync.dma_start(
                    out=out[qc * QC + lt * P : qc * QC + (lt + 1) * P, :],
                    in_=ost[:],
                )

        for qc in range(NQC):
            qs = slice(qc * QC, (qc + 1) * QC)
            attnT_sb = attnp.tile([P, 4, QC], F16, tag="attnT")
            attnT_tiles[qc] = attnT_sb
            for h in range(H):
                hb = (h % 2) * DH  # partition base of head h inside its dm-tile
                ht = h // 2
                ex = exps.tile([P, NKT, QC], F16, tag="ex")
                for ktp in range(NKT // 2):  # pairs of k-tiles share a psum
                    sp = spsum.tile([P, 2 * QC], F32, tag="sp")
                    # bias first (one fp8 DoubleRow matmul per k-tile covers
                    # la*A + ld*fp8(D-5) for all 128 rows and starts the PSUM
                    # region), score accumulates on top and stops
                    for i in range(2):
                        kt = 2 * ktp + i
                        nc.tensor.matmul(
                            sp[:, i * QC : (i + 1) * QC],
                            idents[:, h, :, :], ad_sb[:, kt, :, qs],
                            start=True, stop=False, perf_mode=DR,
                        )
                    for i in range(2):
                        kt = 2 * ktp + i
                        nc.tensor.matmul(
                            sp[:, i * QC : (i + 1) * QC],
                            kt_sb[hb : hb + DH, ht, kt * P : (kt + 1) * P],
                            qt_sb[hb : hb + DH, ht, qs],
                            start=False,
                            stop=True,
                        )
                    nc.scalar.activation(
                        out=ex[:, 2 * ktp : 2 * ktp + 2, :].rearrange(
                            "p a b -> p (a b)"
                        ),
                        in_=sp[:],
                        func=mybir.ActivationFunctionType.Exp,
                    )
                # PV with appended mask column -> row 64 = softmax denominator
                pv = pvwo.tile([P, QC], F32, tag="pvwo")
                for kt in range(NKT):
                    nc.tensor.matmul(
                        pv[0 : DH + 1, :],
                        vx_sb[:, kt, h, :],
                        ex[:, kt, :],
                        start=(kt == 0),
                        stop=(kt == NKT - 1),
                    )
                # normalize: shift denom row to partition 0, fast recip, fp16
                # cast, K=1 ones-matmul broadcast across 64 partitions
                den = small.tile([1, QC], F32, tag="den")
                nc.vector.tensor_copy(out=den[:], in_=pv[DH : DH + 1, :])
                rec = small.tile([1, QC], F32, tag="rec")
                nc.vector.reciprocal_approx_fast(out=rec[:], in_=den[:])
                rec16 = small.tile([1, QC], F16, tag="rec16")
                nc.vector.tensor_copy(out=rec16[:], in_=rec[:])
                bps = bcp.tile([DH, QC], F32, tag="bps")
                nc.tensor.matmul(
                    bps[:],
                    ones_sb[0:1, :],
                    rec16[:],
                    start=True,
                    stop=True,
                )
                pvs = small.tile([DH, QC], F32, tag="bc")
                nc.vector.tensor_copy(out=pvs[:], in_=pv[0:DH, :])
                # inputs share base 0; output base may differ (odd heads land
                # on partitions 64:128 directly)
                nc.vector.tensor_mul(
                    out=attnT_sb[hb : hb + DH, ht, :], in0=pvs[:], in1=bps[:]
                )
                if qc == 1 and h == 0:
                    emit_outproj(0)

        emit_outproj(1)


def build_nc():
    from concourse import bacc

    nc = bacc.Bacc("TRN2", target_bir_lowering=False, debug=False)
    with tile.TileContext(nc) as tc:
        _emit(tc)
    nc.compile()
    return nc


_NC = None


def _get_nc():
    global _NC
    if _NC is None:
        _NC = build_nc()
    return _NC


def make_in_maps(queries, keys, values, attention_mask, adjacency_matrix,
                 distance_matrix, W_q, W_k, W_v, W_o, lambda_a, lambda_d):
    import ml_dtypes

    f = np.float32
    h16 = np.float16
    f8 = ml_dtypes.float8_e4m3
    c = np.ascontiguousarray
    wqT = c((W_q.astype(f) * f(0.125)).T).astype(h16)
    wkT = c(W_k.astype(f).T).astype(h16)
    wvT = c(W_v.astype(f).T).astype(h16)
    woT = c(W_o.astype(f).T).astype(h16)
    la8 = lambda_a.astype(f).astype(f8).astype(f)
    ld8 = lambda_d.astype(f).astype(f8).astype(f)
    idn = np.zeros((P, H, 2, P), dtype=f)
    rr = np.arange(P)
    for h in range(H):
        idn[rr, h, 0, rr] = la8[h]
        idn[rr, h, 1, rr] = ld8[h]
    idn8 = idn.reshape(P, H * 2 * P).astype(f8)
    in_maps = []
    for b in range(B):
        # per k-tile block of 128 rows: [A | fp8(D-5)]; the -5 shift centers
        # D's fp8 range and cancels in softmax
        A8 = adjacency_matrix[b].astype(f).T.astype(f8)
        Dhi = (distance_matrix[b].astype(f).T - f(5.0)).astype(f8)
        ad = np.concatenate(
            [A8.reshape(NKT, P, L), Dhi.reshape(NKT, P, L)], axis=2
        )  # [NKT, P, 2L]
        in_maps.append({
            "qT": c(queries[b].astype(f).T).astype(h16),
            "kT": c(keys[b].astype(f).T).astype(h16),
            "vT": c(values[b].astype(f).T).astype(h16),
            "wqT": wqT, "wkT": wkT, "wvT": wvT, "woT": woT,
            "ad8": c(ad.reshape(NKT * P, 2 * L)),
            "mask01": c((attention_mask[b] > 0).astype(f).reshape(NKT, P).T),
            "idn8": idn8,
        })
    return in_maps


def kernel(queries, keys, values, attention_mask, adjacency_matrix,
           distance_matrix, W_q, W_k, W_v, W_o, lambda_a, lambda_d, **kw):
    nc = _get_nc()
    in_maps = make_in_maps(queries, keys, values, attention_mask,
                           adjacency_matrix, distance_matrix,
                           W_q, W_k, W_v, W_o, lambda_a, lambda_d)
    res = run_bass_kernel_spmd(nc, in_maps, list(range(B)), **kw)
    outs = np.stack([res.results[i]["out"] for i in range(B)]).astype(np.float32)
    return outs


# revision 42
# speedup vs baseline: 1.0604x; 1.0173x over previous
"""MSRSA multi-head attention kernel for 8 Trainium2 NeuronCores.

Strategy: data-parallel over batch (B=8 -> 1 batch element per core).
Per core, for its batch element b:
  Qt = (W_q/8) @ queries^T        [512,1024]  (scale 1/8 folded into W_q)
  Kt = W_k @ keys^T               [512,1024]
  V  = values @ W_v^T             [1024,512]  (rows masked by attention_mask)
  per head h, scores are computed TRANSPOSED: S_T[k,q]:
     S_T = sum_d Kt[d,k]*Qt[d,q] + la[h]*A^T[k,q] + ld[h]*D^T[k,q]
  (A/D bias injected by scaled-identity matmuls accumulating into PSUM)
  expS = exp(S_T) on ScalarE (PSUM -> SBUF evacuation is the exp)
  attnT_h[d,q] (+ denominator row) = sum_k V_ext[k, d|mask] * expS[k,q]
  (mask column of V_ext -> row 64 of PV output = softmax denominator)
  normalize: denom row copied to partition 0 (copies may retarget the output
  base), reciprocal_approx_fast at base 0, fp16 cast, K=1 ones-matmul
  broadcast; the multiply writes even heads at partitions 0:64 and odd heads
  at 64:128 directly.
  out = attnT contracted with W_o^T   [1024, 512] (fp16, host upcasts)

On TRN2 a matmul costs ~N output columns at 1 col/cycle regardless of dtype
or contraction size, so the kernel minimizes matmul COUNT and keeps weight
loads small (fp16 64-col ident tiles) so LDWEIGHTS hides under execution.
"""

import contextlib

import numpy as np

import concourse.bass as bass
import concourse.mybir as mybir
import concourse.tile as tile
from concourse.bass_utils import run_bass_kernel_spmd

B, L, DIN, DM, H = 8, 1024, 256, 512, 8
DH = DM // H  # 64
P = 128
NKT = L // P          # 8 k-tiles
NQC = 2               # q chunks
QC = L // NQC         # 512
F32 = mybir.dt.float32
F16 = mybir.dt.float16
F8 = mybir.dt.float8e4
DR = mybir.MatmulPerfMode.DoubleRow


def _emit(tc):
    nc = tc.nc

    def dram(name, shape, dtype=F16, kind="ExternalInput"):
        return nc.dram_tensor(name, shape, dtype, kind=kind).ap()

    qT = dram("qT", [DIN, L])
    kT = dram("kT", [DIN, L])
    vT = dram("vT", [DIN, L])
    wqT = dram("wqT", [DIN, DM])
    wkT = dram("wkT", [DIN, DM])
    wvT = dram("wvT", [DIN, DM])
    woT = dram("woT", [DM, DM])
    ad8 = dram("ad8", [NKT * P, 2 * L], F8)  # per row: [A | fp8(D-5)]
    idn8 = dram("idn8", [P, H * 2 * P], F8)  # per-head (la, ld) diag subtiles
    mask01 = dram("mask01", [P, NKT], F32)
    out = dram("out", [L, DM], F16, kind="ExternalOutput")

    with contextlib.ExitStack() as ctx:
        singles = ctx.enter_context(tc.tile_pool(name="singles", bufs=1))
        big = ctx.enter_context(tc.tile_pool(name="big", bufs=1))
        exps = ctx.enter_context(tc.tile_pool(name="exps", bufs=4))
        small = ctx.enter_context(tc.tile_pool(name="small", bufs=3))
        attnp = ctx.enter_context(tc.tile_pool(name="attnp", bufs=2))
        spsum = ctx.enter_context(tc.tile_pool(name="spsum", bufs=2, space="PSUM"))
        pvwo = ctx.enter_context(tc.tile_pool(name="pvwo", bufs=3, space="PSUM"))
        bcp = ctx.enter_context(tc.tile_pool(name="bcp", bufs=1, space="PSUM"))

        # ---- big SBUF-resident tensors ----
        ad_sb = big.tile([P, NKT, 2, L], F8, tag="ad")  # [A | Dhi] k-tile rows
        qt_sb = big.tile([P, 4, L], F16, tag="qt")       # [p,t,l] = Qt[t*128+p, l]
        kt_sb = big.tile([P, 4, L], F16, tag="kt")
        vx_sb = big.tile([P, NKT, H, DH + 1], F16, tag="vx")  # V + mask column

        ad8_r = ad8.rearrange("(t p) (j q) -> p t j q", p=P, q=L)

        # ---- phase 1: projections (pools scoped so SBUF is reclaimed) ----
        proj_ctx = contextlib.ExitStack()
        stage = proj_ctx.enter_context(tc.tile_pool(name="stage", bufs=3))
        wpool = proj_ctx.enter_context(tc.tile_pool(name="wpool", bufs=3))

        def load_stage(src, eng):
            t = stage.tile([P, 2, L], F16, tag="stage")
            r = src.rearrange("(t p) l -> p t l", p=P)
            for i in range(2):  # per-half DMAs so the first matmul starts early
                eng.dma_start(out=t[:, i, :], in_=r[:, i, :])
            return t

        def load_w(src, eng):
            t = wpool.tile([P, 2, DM], F16, tag="w")
            r = src.rearrange("(t p) d -> p t d", p=P)
            for i in range(2):
                eng.dma_start(out=t[:, i, :], in_=r[:, i, :])
            return t

        # DMA issue order = dependency order: Q/K paths gate the first
        # matmuls, idents+ad gate the first bias matmul, V/Wo come later.
        # Issue across both HWDGE engines (sync + scalar) so descriptor
        # generation is not serialized at the head of the kernel.
        q_sb, wq_sb = load_stage(qT, nc.sync), load_w(wqT, nc.scalar)
        k_sb, wk_sb = load_stage(kT, nc.sync), load_w(wkT, nc.scalar)

        idents = singles.tile([P, H, 2, P], F8, tag="idents")
        nc.scalar.dma_start(
            out=idents[:], in_=idn8.rearrange("p (h j m) -> p h j m", h=H, j=2)
        )
        for t in range(NKT):
            eng = nc.sync if t % 2 == 0 else nc.scalar
            eng.dma_start(out=ad_sb[:, t, :, :], in_=ad8_r[:, t, :, :])

        mask_sb = singles.tile([P, NKT], F32, tag="mask")
        nc.scalar.dma_start(out=mask_sb[:], in_=mask01[:])
        ones_sb = singles.tile([P, DH], F16, tag="ones")
        nc.vector.memset(ones_sb[:], 1.0)

        v_sb, wv_sb = load_stage(vT, nc.sync), load_w(wvT, nc.scalar)
        wo_sb = singles.tile([P, 4, DM], F16, tag="wo")
        nc.sync.dma_start(out=wo_sb[:], in_=woT.rearrange("(t p) d -> p t d", p=P))

        # Qt / Kt: out[m=dm-tile, n=l-chunk] = sum_din w?T[din, dm] * xT[din, l]
        for x_sb, w_sb, dst in ((q_sb, wq_sb, qt_sb), (k_sb, wk_sb, kt_sb)):
            for mt in range(4):
                for lc in range(NQC):
                    ps = pvwo.tile([P, QC], F32, tag="pvwo")
                    for kt2 in range(2):
                        nc.tensor.matmul(
                            ps[:],
                            w_sb[:, kt2, mt * P : (mt + 1) * P],
                            x_sb[:, kt2, lc * QC : (lc + 1) * QC],
                            start=(kt2 == 0),
                            stop=(kt2 == 1),
                        )
                    nc.vector.tensor_copy(
                        out=dst[:, mt, lc * QC : (lc + 1) * QC], in_=ps[:]
                    )

        # V: out[m=l-tile, n=dm] = sum_din vT[din, l] * wvT[din, dm]; mask rows
        for lt in range(NKT):
            ps = pvwo.tile([P, DM], F32, tag="pvwo")
            for kt2 in range(2):
                nc.tensor.matmul(
                    ps[:],
                    v_sb[:, kt2, lt * P : (lt + 1) * P],
                    wv_sb[:, kt2, :],
                    start=(kt2 == 0),
                    stop=(kt2 == 1),
                )
            nc.vector.tensor_scalar_mul(
                out=vx_sb[:, lt, :, 0:DH],
                in0=ps.rearrange("p (h d) -> p h d", h=H),
                scalar1=mask_sb[:, lt : lt + 1],
            )
            # mask column (softmax denominator counts only unmasked keys)
            nc.vector.tensor_copy(
                out=vx_sb[:, lt, :, DH : DH + 1],
                in_=mask_sb[:, lt : lt + 1, None].to_broadcast((P, H, 1)),
            )

        proj_ctx.close()

        # ---- phase 2: attention ----
        # qc0's output projection is emitted after qc1's first head so the PE
        # has score work queued while the last head's normalization (vector
        # side) completes — attnT is double-buffered to allow it.
        attnT_tiles = {}

        def emit_outproj(qc):
            attn = attnT_tiles[qc]
            for lt in range(QC // P):
                ws = pvwo.tile([P, DM], F32, tag="pvwo")
                for kt4 in range(4):
                    nc.tensor.matmul(
                        ws[:],
                        attn[:, kt4, lt * P : (lt + 1) * P],
                        wo_sb[:, kt4, :],
                        start=(kt4 == 0),
                        stop=(kt4 == 3),
                    )
                ost = small.tile([P, DM], F16, tag="ost")
                nc.scalar.copy(out=ost[:], in_=ws[:])
                nc.sync.dma_start(
                    out=out[qc * QC + lt * P : qc * QC + (lt + 1) * P, :],
                    in_=ost[:],
                )

        for qc in range(NQC):
            qs = slice(qc * QC, (qc + 1) * QC)
            attnT_sb = attnp.tile([P, 4, QC], F16, tag="attnT")
            attnT_tiles[qc] = attnT_sb
            for h in range(H):
                hb = (h % 2) * DH  # partition base of head h inside its dm-tile
                ht = h // 2
                ex = exps.tile([P, NKT, QC], F16, tag="ex")
                for ktp in range(NKT // 2):  # pairs of k-tiles share a psum
                    sp = spsum.tile([P, 2 * QC], F32, tag="sp")
                    # bias first (one fp8 DoubleRow matmul per k-tile covers
                    # la*A + ld*fp8(D-5) for all 128 rows and starts the PSUM
                    # region), score accumulates on top and stops
                    for i in range(2):
                        kt = 2 * ktp + i
                        nc.tensor.matmul(
                            sp[:, i * QC : (i + 1) * QC],
                            idents[:, h, :, :], ad_sb[:, kt, :, qs],
                            start=True, stop=False, perf_mode=DR,
                        )
                    for i in range(2):
                        kt = 2 * ktp + i
                        nc.tensor.matmul(
                            sp[:, i * QC : (i + 1) * QC],
                            kt_sb[hb : hb + DH, ht, kt * P : (kt + 1) * P],
                            qt_sb[hb : hb + DH, ht, qs],
                            start=False,
                            stop=True,
                        )
                    nc.scalar.activation(
                        out=ex[:, 2 * ktp : 2 * ktp + 2, :].rearrange(
                            "p a b -> p (a b)"
                        ),
                        in_=sp[:],
                        func=mybir.ActivationFunctionType.Exp,
                    )
                # PV with appended mask column -> row 64 = softmax denominator
                pv = pvwo.tile([P, QC], F32, tag="pvwo")
                for kt in range(NKT):
                    nc.tensor.matmul(
                        pv[0 : DH + 1, :],
                        vx_sb[:, kt, h, :],
                        ex[:, kt, :],
                        start=(kt == 0),
                        stop=(kt == NKT - 1),
                    )
                # normalize: shift denom row to partition 0, fast recip, fp16
                # cast, K=1 ones-matmul broadcast across 64 partitions
                den = small.tile([1, QC], F32, tag="den")
                nc.vector.tensor_copy(out=den[:], in_=pv[DH : DH + 1, :])
                rec = small.tile([1, QC], F32, tag="rec")
                nc.vector.reciprocal_approx_fast(out=rec[:], in_=den[:])
                rec16 = small.tile([1, QC], F16, tag="rec16")
                nc.vector.tensor_copy(out=rec16[:], in_=rec[:])
                bps = bcp.tile([DH, QC], F32, tag="bps")
                nc.tensor.matmul(
                    bps[:],
                    ones_sb[0:1, :],
                    rec16[:],
                    start=True,
                    stop=True,
                )
                pvs = small.tile([DH, QC], F32, tag="bc")
                nc.vector.tensor_copy(out=pvs[:], in_=pv[0:DH, :])
                # inputs share base 0; output base may differ (odd heads land
                # on partitions 64:128 directly)
                nc.vector.tensor_mul(
                    out=attnT_sb[hb : hb + DH, ht, :], in0=pvs[:], in1=bps[:]
                )
                if qc == 1 and h == 0:
                    emit_outproj(0)

        emit_outproj(1)


def build_nc():
    from concourse import bacc

    nc = bacc.Bacc("TRN2", target_bir_lowering=False, debug=False)
    with tile.TileContext(nc) as tc:
        _emit(tc)
    nc.compile()
    return nc


_NC = None


def _get_nc():
    global _NC
    if _NC is None:
        _NC = build_nc()
    return _NC


def make_in_maps(queries, keys, values, attention_mask, adjacency_matrix,
                 distance_matrix, W_q, W_k, W_v, W_o, lambda_a, lambda_d):
    import ml_dtypes

    f = np.float32
    h16 = np.float16
    f8 = ml_dtypes.float8_e4m3
    c = np.ascontiguousarray
    wqT = c((W_q.astype(f) * f(0.125)).T).astype(h16)
    wkT = c(W_k.astype(f).T).astype(h16)
    wvT = c(W_v.astype(f).T).astype(h16)
    woT = c(W_o.astype(f).T).astype(h16)
    la8 = lambda_a.astype(f).astype(f8).astype(f)
    ld8 = lambda_d.astype(f).astype(f8).astype(f)
    idn = np.zeros((P, H, 2, P), dtype=f)
    rr = np.arange(P)
    for h in range(H):
        idn[rr, h, 0, rr] = la8[h]
        idn[rr, h, 1, rr] = ld8[h]
    idn8 = idn.reshape(P, H * 2 * P).astype(f8)
    in_maps = []
    for b in range(B):
        # per k-tile block of 128 rows: [A | fp8(D-5)]; the -5 shift centers
        # D's fp8 range and cancels in softmax
        A8 = adjacency_matrix[b].astype(f).T.astype(f8)
        Dhi = (distance_matrix[b].astype(f).T - f(5.0)).astype(f8)
        ad = np.concatenate(
            [A8.reshape(NKT, P, L), Dhi.reshape(NKT, P, L)], axis=2
        )  # [NKT, P, 2L]
        in_maps.append({
            "qT": c(queries[b].astype(f).T).astype(h16),
            "kT": c(keys[b].astype(f).T).astype(h16),
            "vT": c(values[b].astype(f).T).astype(h16),
            "wqT": wqT, "wkT": wkT, "wvT": wvT, "woT": woT,
            "ad8": c(ad.reshape(NKT * P, 2 * L)),
            "mask01": c((attention_mask[b] > 0).astype(f).reshape(NKT, P).T),
            "idn8": idn8,
        })
    return in_maps


def kernel(queries, keys, values, attention_mask, adjacency_matrix,
           distance_matrix, W_q, W_k, W_v, W_o, lambda_a, lambda_d, **kw):
    nc = _get_nc()
    in_maps = make_in_maps(queries, keys, values, attention_mask,
                           adjacency_matrix, distance_matrix,
                           W_q, W_k, W_v, W_o, lambda_a, lambda_d)
    res = run_bass_kernel_spmd(nc, in_maps, list(range(B)), **kw)
    outs = np.stack([res.results[i]["out"] for i in range(B)]).astype(np.float32)
    return outs
